# revision 15
# baseline (speedup 1.0000x reference)
"""BinaryLinear kernel for Trainium2 (8 NeuronCores, SPMD).

Computes y = x @ sign(W)^T + sign(b) with x:[8192,4096] f32,
W:[4096,4096] f32, b:[4096] f32.

Sharding: 2-way over tokens x 4-way over out_features (8 cores).
Per core: x_shard [4096, 4096], W_shard [1024, 4096], b_shard [1024]
-> y_shard [4096, 1024]. No collectives; host shards/concats.

Strategy (v2, fp8 DoubleRow hybrid):
  - sign(W) is exactly representable in fp8; the only precision loss is
    quantizing x. e4m3(x) alone gives rel err 2.12e-2 (gate 2e-2), so
    run a hybrid: KQ k-slabs as e4m3 via DoubleRow matmuls (2 slabs
    per MM, 2 MACs/cell/cycle) + H k-slabs as bf16 normal matmuls.
    Host-emulated exact rel err (== HW to 6 digits, verified): H=8
    1.70e-2, H=4 1.87e-2 vs the 2e-2 gate.
  - DoubleRow MM: lhsT = x pair [128, 2, 128] fp8, rhs = sign(W)^T pair
    [128, 2, 512] fp8, out [128 tokens, 512 outs] f32 PSUM,
    d = w0*m0 + w1*m1 per cell (contraction 256/column-cycle).
  - Layouts are the baseline's tile-major packs, split into an fp8 part
    (slabs 0..23) and a bf16 part (slabs 24..31). Sign via DVE bit
    tricks: fp8 (w & 0x80) | 0x38 done 4-at-a-time on u32 views; bf16
    (w & 0x8000) | 0x3F80 on u16; bias u32 trick.
  - Single SWDGE (gpsimd) load queue in bandwidth-priority order, same
    chase scheme as baseline: CHASE leading token tiles slab-interleave
    their MM emission to chase W-chunk arrival; y stores on the scalar
    HWDGE queue; last tile runs group-major in 256-wide strips.
"""

import sys

sys.path.insert(0, "/opt/trn_rl_repo")

from contextlib import ExitStack

import numpy as np

import concourse.bass as bass  # noqa: F401
import concourse.mybir as mybir
from concourse import bacc, tile
from concourse.bass_utils import run_bass_kernel_spmd

TOKENS, IN, OUT = 8192, 4096, 4096
N_CORES = 8
T_SPLIT, O_SPLIT = 2, 4
T_CORE, O_CORE = TOKENS // T_SPLIT, OUT // O_SPLIT

P = 128
FREE = 512  # matmul moving free dim / psum bank width (f32)
H = 2  # k-slabs computed in bf16 (accuracy top-up); rest fp8 DoubleRow
KQ = 32 - H  # fp8 k-slabs (must be even)
KP = KQ // 2  # DoubleRow slab-pairs
CHASE = 3  # leading token tiles that unit-interleave to chase W
TRAIL = 2  # chase tile i trails tile i-1 by this many units in PE order
XBUF = 4  # x tile buffer depth
XSPLIT = 16  # fp8 k-slabs in each chase tile's head DMA

F32 = mybir.dt.float32
BF16 = mybir.dt.bfloat16
FP8 = mybir.dt.float8e4
U16 = mybir.dt.uint16
U32 = mybir.dt.uint32

DR = mybir.MatmulPerfMode.DoubleRow

# per-tile PE units: units 0..KP-1 are DoubleRow pairs (slabs 2u,2u+1),
# units KP..KP+H-1 are bf16 slabs (slab u-KP of the bf16 part)
UNITS = KP + H


def emit(nc, tc, xq_d, xb_d, wq_d, wb_d, b_d, y_d, t_core, o_core):
    """Per-core program.
    xq_d [t_core, KQ*P] fp8 = x fp8 part packed tile-major:
        xq[tt*128 + p, ks*128 + t] = e4m3(x[tt*128 + t, ks*128 + p])
    xb_d [t_core, H*P] bf16 = x bf16 part, slabs KQ..31:
        xb[tt*128 + p, j*128 + t] = bf16(x[tt*128 + t, (KQ+j)*128 + p])
    wq_d [128, KQ*o_core] fp8 = W^T fp8 part partition-major:
        wq[p, ks*o_core + o] = e4m3(W[o, ks*128 + p]) (sign-exact)
    wb_d [128, H*o_core] bf16 same for slabs KQ..31.
    b_d [1, o_core] f32, y_d [t_core, o_core] f32."""
    TT = t_core // P
    OG = o_core // FREE

    with ExitStack() as ctx:
        const = ctx.enter_context(tc.tile_pool(name="const", bufs=1))
        swq = const.tile([P, KQ, o_core], FP8)  # resident sign(W)^T fp8
        swb = const.tile([P, H, o_core], BF16)  # resident sign(W)^T bf16
        bias_bc = const.tile([P, o_core], F32)

        xqpool = ctx.enter_context(tc.tile_pool(name="xqload", bufs=XBUF))
        xbpool = ctx.enter_context(tc.tile_pool(name="xbload", bufs=XBUF))
        psum = ctx.enter_context(tc.tile_pool(name="psum", bufs=8, space="PSUM"))
        opool = ctx.enter_context(tc.tile_pool(name="yout", bufs=3))

        # ---- SWDGE load queue, in bandwidth-priority order ----
        def load_tile(tt):
            xq = xqpool.tile([P, KQ, P], FP8, name="xq")
            nc.gpsimd.dma_start(xq, xq_d[tt * P : (tt + 1) * P, :])
            xb = xbpool.tile([P, H, P], BF16, name="xb")
            nc.gpsimd.dma_start(xb, xb_d[tt * P : (tt + 1) * P, :])
            return (xq, xb)

        def load_wq_chunk(c0, w, eng=None):
            # DMA straight into the resident table, then DVE u32
            # bit-trick sign in place, 4 fp8 lanes at a time:
            # (w & 0x80) | 0x38 == +-1.0 e4m3. No staging buffer: no
            # slot reuse, so no WAR window on the load queue.
            dst = swq[:, c0 : c0 + w, :]
            (eng or nc.gpsimd).dma_start(dst, wq_d[:, c0 * o_core : (c0 + w) * o_core])
            nc.vector.tensor_scalar(
                out=dst.bitcast(U32),
                in0=dst.bitcast(U32),
                scalar1=0x80808080,
                scalar2=0x38383838,
                op0=mybir.AluOpType.bitwise_and,
                op1=mybir.AluOpType.bitwise_or,
            )

        def load_wb_chunk(c0, w):
            # (w & 0x8000) | 0x3F80 == +-1.0 bf16, in place as above
            dst = swb[:, c0 : c0 + w, :]
            nc.gpsimd.dma_start(dst, wb_d[:, c0 * o_core : (c0 + w) * o_core])
            nc.vector.tensor_scalar(
                out=dst.bitcast(U16),
                in0=dst.bitcast(U16),
                scalar1=0x8000,
                scalar2=0x3F80,
                op0=mybir.AluOpType.bitwise_and,
                op1=mybir.AluOpType.bitwise_or,
            )

        # Chase tiles split their fp8 x into a head DMA (slabs
        # 0..XSPLIT-1, a contiguous prefix of the packed row) + the
        # rest; W chunks interleave so the strict-FIFO PE queue always
        # has signed slabs AND x data by the time it reaches each MM.
        # The first two W pair-chunks ride the (otherwise idle until
        # first eviction) scalar HWDGE queue, in parallel with the x
        # heads on gpsimd, so the first MMs aren't serialized behind
        # the whole cold queue.
        xqh = [xqpool.tile([P, KQ, P], FP8, name="xq") for _ in range(CHASE)]
        xbh = [xbpool.tile([P, H, P], BF16, name="xb") for _ in range(CHASE)]
        XS2 = XSPLIT // 2
        load_wq_chunk(0, 2, eng=nc.scalar)  # unit 0
        nc.gpsimd.dma_start(xqh[0][:, :XS2, :], xq_d[0:P, : XS2 * P])
        load_wq_chunk(2, 2, eng=nc.scalar)  # unit 1
        nc.gpsimd.dma_start(xqh[1][:, :XS2, :], xq_d[P : 2 * P, : XS2 * P])
        nc.gpsimd.dma_start(xqh[0][:, XS2:XSPLIT, :], xq_d[0:P, XS2 * P : XSPLIT * P])
        nc.gpsimd.dma_start(xqh[1][:, XS2:XSPLIT, :], xq_d[P : 2 * P, XS2 * P : XSPLIT * P])
        for t in range(2, CHASE):
            nc.gpsimd.dma_start(xqh[t][:, :XSPLIT, :], xq_d[t * P : (t + 1) * P, : XSPLIT * P])
        load_wq_chunk(4, 4)  # units 2,3
        load_wq_chunk(8, 4)  # units 4,5
        load_wq_chunk(12, 4)  # units 6,7
        for t in range(CHASE):
            nc.gpsimd.dma_start(xqh[t][:, XSPLIT:, :], xq_d[t * P : (t + 1) * P, XSPLIT * P :])
        c0 = 16
        while c0 < KQ:
            w = min(4, KQ - c0)
            load_wq_chunk(c0, w)
            c0 += w
        for t in range(CHASE):
            nc.gpsimd.dma_start(xbh[t], xb_d[t * P : (t + 1) * P, :])
        load_wb_chunk(0, H)  # last H units

        # next tile ahead of bias on the load queue
        xnext_tile = load_tile(CHASE)
        nc.gpsimd.dma_start(bias_bc, b_d.to_broadcast([P, o_core]))
        nc.vector.tensor_scalar(
            out=bias_bc.bitcast(U32),
            in0=bias_bc.bitcast(U32),
            scalar1=0x80000000,
            scalar2=0x3F800000,
            op0=mybir.AluOpType.bitwise_and,
            op1=mybir.AluOpType.bitwise_or,
        )

        # ---- compute ----
        def evict_store(tt, pss, seng=None):
            seng = seng or nc.scalar
            yo = opool.tile([P, o_core], F32, name="yo")
            for og in range(OG):
                ocol = slice(og * FREE, (og + 1) * FREE)
                nc.vector.tensor_tensor(
                    out=yo[:, ocol],
                    in0=pss[og],
                    in1=bias_bc[:, ocol],
                    op=mybir.AluOpType.add,
                )
                seng.dma_start(y_d[tt * P : (tt + 1) * P, ocol], yo[:, ocol])

        def mm_unit(pss, xpair, u):
            xq3, xb3 = xpair
            if u < KP:
                for og in range(OG):
                    nc.tensor.matmul(
                        pss[og],
                        xq3[:, 2 * u : 2 * u + 2, :],
                        swq[:, 2 * u : 2 * u + 2, og * FREE : (og + 1) * FREE],
                        start=(u == 0),
                        stop=False,
                        perf_mode=DR,
                    )
            else:
                j = u - KP
                for og in range(OG):
                    nc.tensor.matmul(
                        pss[og],
                        xb3[:, j, :],
                        swb[:, j, og * FREE : (og + 1) * FREE],
                        start=False,
                        stop=(j == H - 1),
                    )

        # tiles 0..CHASE-1 chase W arrival, each trailing the previous
        # by TRAIL units so later tiles never head-of-line-block the
        # strict FIFO PE queue on their x DMAs
        ch_ps = [
            [psum.tile([P, FREE], F32, name="ps") for _ in range(OG)]
            for _ in range(CHASE)
        ]
        for u in range(UNITS + TRAIL * (CHASE - 1)):
            for t in range(CHASE):
                uu = u - TRAIL * t
                if 0 <= uu < UNITS:
                    mm_unit(ch_ps[t], (xqh[t], xbh[t]), uu)
        for t in range(CHASE):
            evict_store(t, ch_ps[t])

        # steady state: per-tile loads, prefetched XBUF deep. The last
        # tile runs og-major so group 0 evicts+stores under group 1's MMs
        def do_tile(tt, xpair):
            xq3, xb3 = xpair
            if tt == TT - 1:
                # group-major in 256-wide strips: each strip evicts and
                # stores (128 KB) under the next strip's MMs, minimizing
                # the post-last-MM tail
                G = 256
                yo = opool.tile([P, o_core], F32, name="yo")
                for g in range(o_core // G):
                    ocol = slice(g * G, (g + 1) * G)
                    ps = psum.tile([P, FREE], F32, name="ps")
                    for u in range(KP):
                        nc.tensor.matmul(
                            ps[:, :G],
                            xq3[:, 2 * u : 2 * u + 2, :],
                            swq[:, 2 * u : 2 * u + 2, ocol],
                            start=(u == 0),
                            stop=False,
                            perf_mode=DR,
                        )
                    for j in range(H):
                        nc.tensor.matmul(
                            ps[:, :G],
                            xb3[:, j, :],
                            swb[:, j, ocol],
                            start=False,
                            stop=(j == H - 1),
                        )
                    nc.vector.tensor_tensor(
                        out=yo[:, ocol],
                        in0=ps[:, :G],
                        in1=bias_bc[:, ocol],
                        op=mybir.AluOpType.add,
                    )
                    # the idle Sync-engine HWDGE queue skips the scalar
                    # queue's backlog for the exposed final stores (the
                    # gpsimd SWDGE queue is NOT suitable: a recently
                    # active Q7 queue costs ~8us in the epilogue DRAIN)
                    nc.sync.dma_start(y_d[tt * P : (tt + 1) * P, ocol], yo[:, ocol])
                return
            pss = [psum.tile([P, FREE], F32, name="ps") for _ in range(OG)]
            for u in range(UNITS):
                mm_unit(pss, xpair, u)
            evict_store(tt, pss, seng=nc.sync if tt >= TT - 2 else None)

        pending = {CHASE: xnext_tile}
        for t in range(CHASE + 1, min(CHASE + XBUF, TT)):
            pending[t] = load_tile(t)
        for tt in range(CHASE, TT):
            nxt = tt + XBUF
            if nxt < TT:
                pending[nxt] = load_tile(nxt)
            do_tile(tt, pending.pop(tt))


def build(t_core=T_CORE, o_core=O_CORE):
    nc = bacc.Bacc("TRN2", target_bir_lowering=False, debug=False)
    xq_d = nc.dram_tensor("xq", [t_core, KQ * P], FP8, kind="ExternalInput")
    xb_d = nc.dram_tensor("xb", [t_core, H * P], BF16, kind="ExternalInput")
    wq_d = nc.dram_tensor("wq", [P, KQ * o_core], FP8, kind="ExternalInput")
    wb_d = nc.dram_tensor("wb", [P, H * o_core], BF16, kind="ExternalInput")
    b_d = nc.dram_tensor("b", [1, o_core], F32, kind="ExternalInput")
    y_d = nc.dram_tensor("y", [t_core, o_core], F32, kind="ExternalOutput")
    with tile.TileContext(nc) as tc:
        emit(nc, tc, xq_d.ap(), xb_d.ap(), wq_d.ap(), wb_d.ap(), b_d.ap(), y_d.ap(),
             t_core, o_core)
    nc.compile()
    return nc


_nc_cache = None


def _pack_x_shard(x_sh, dt, k0, k1):
    """[t_core, in] slabs k0..k1-1 -> xp[tt*128+p, ks*128+t] = dt(x[tt*128+t, (k0+ks)*128+p])"""
    t_core = x_sh.shape[0]
    a = x_sh[:, k0 * P : k1 * P].astype(dt)
    a = a.reshape(t_core // P, P, k1 - k0, P)  # [tt, t, ks, p]
    return np.ascontiguousarray(a.transpose(0, 3, 2, 1)).reshape(t_core, -1)


def _pack_w_shard(w_sh, dt, k0, k1):
    """[o_core, in] slabs k0..k1-1 -> wt[p, ks*o_core+o] = dt(W[o, (k0+ks)*128+p])"""
    o_core = w_sh.shape[0]
    a = w_sh[:, k0 * P : k1 * P].T.astype(dt)  # [(k1-k0)*P, o_core]
    a = a.reshape(k1 - k0, P, o_core)  # [ks, p, o]
    return np.ascontiguousarray(a.transpose(1, 0, 2)).reshape(P, -1)


def kernel(x: np.ndarray, weight: np.ndarray, bias: np.ndarray, **run_kwargs):
    global _nc_cache
    if _nc_cache is None:
        _nc_cache = build()
    nc = _nc_cache

    import ml_dtypes

    bf16 = ml_dtypes.bfloat16
    fp8 = ml_dtypes.float8_e4m3
    x = np.ascontiguousarray(x, dtype=np.float32)
    weight = np.ascontiguousarray(weight, dtype=np.float32)
    bias = np.ascontiguousarray(bias, dtype=np.float32)

    xq_shards = [
        _pack_x_shard(x[th * T_CORE : (th + 1) * T_CORE], fp8, 0, KQ)
        for th in range(T_SPLIT)
    ]
    xb_shards = [
        _pack_x_shard(x[th * T_CORE : (th + 1) * T_CORE], bf16, KQ, 32)
        for th in range(T_SPLIT)
    ]
    # fp8/bf16 casts are sign-bit-exact; only sign(W) is consumed on-device
    wq_shards = [
        _pack_w_shard(weight[oq * O_CORE : (oq + 1) * O_CORE], fp8, 0, KQ)
        for oq in range(O_SPLIT)
    ]
    wb_shards = [
        _pack_w_shard(weight[oq * O_CORE : (oq + 1) * O_CORE], bf16, KQ, 32)
        for oq in range(O_SPLIT)
    ]

    in_maps = []
    for c in range(N_CORES):
        th, oq = divmod(c, O_SPLIT)
        in_maps.append(
            {
                "xq": xq_shards[th],
                "xb": xb_shards[th],
                "wq": wq_shards[oq],
                "wb": wb_shards[oq],
                "b": bias[oq * O_CORE : (oq + 1) * O_CORE].reshape(1, O_CORE),
            }
        )
    res = run_bass_kernel_spmd(nc, in_maps, core_ids=list(range(N_CORES)), **run_kwargs)
    y = np.empty((TOKENS, OUT), dtype=np.float32)
    for c in range(N_CORES):
        th, oq = divmod(c, O_SPLIT)
        y[th * T_CORE : (th + 1) * T_CORE, oq * O_CORE : (oq + 1) * O_CORE] = (
            res.results[c]["y"]
        )
    kernel.last_results = res
    return y


# revision 19
# speedup vs baseline: 1.1957x; 1.1957x over previous
"""BinaryLinear kernel for Trainium2 (8 NeuronCores, SPMD).

Computes y = x @ sign(W)^T + sign(b) with x:[8192,4096] f32,
W:[4096,4096] f32, b:[4096] f32.

Sharding: 2-way over tokens x 4-way over out_features (8 cores).
Per core: x_shard [4096, 4096], W_shard [1024, 4096], b_shard [1024]
-> y_shard [4096, 1024]. No collectives; host shards/concats.

Strategy (v2, fp8 DoubleRow hybrid):
  - sign(W) is exactly representable in fp8; the only precision loss is
    quantizing x. e4m3(x) alone gives rel err 2.12e-2 (gate 2e-2), so
    run a hybrid: KQ k-slabs as e4m3 via DoubleRow matmuls (2 slabs
    per MM, 2 MACs/cell/cycle) + H k-slabs as bf16 normal matmuls.
    Host-emulated exact rel err (== HW to 6 digits, verified): H=8
    1.70e-2, H=4 1.87e-2 vs the 2e-2 gate.
  - DoubleRow MM: lhsT = x pair [128, 2, 128] fp8, rhs = sign(W)^T pair
    [128, 2, 512] fp8, out [128 tokens, 512 outs] f32 PSUM,
    d = w0*m0 + w1*m1 per cell (contraction 256/column-cycle).
  - Layouts are the baseline's tile-major packs, split into an fp8 part
    (slabs 0..23) and a bf16 part (slabs 24..31). Sign via DVE bit
    tricks: fp8 (w & 0x80) | 0x38 done 4-at-a-time on u32 views; bf16
    (w & 0x8000) | 0x3F80 on u16; bias u32 trick.
  - Single SWDGE (gpsimd) load queue in bandwidth-priority order, same
    chase scheme as baseline: CHASE leading token tiles slab-interleave
    their MM emission to chase W-chunk arrival; y stores on the scalar
    HWDGE queue; last tile runs group-major in 256-wide strips.
"""

import sys

sys.path.insert(0, "/opt/trn_rl_repo")

from contextlib import ExitStack

import numpy as np

import concourse.bass as bass  # noqa: F401
import concourse.mybir as mybir
from concourse import bacc, tile
from concourse.bass_utils import run_bass_kernel_spmd

TOKENS, IN, OUT = 8192, 4096, 4096
N_CORES = 8
T_SPLIT, O_SPLIT = 2, 4
T_CORE, O_CORE = TOKENS // T_SPLIT, OUT // O_SPLIT

P = 128
FREE = 512  # matmul moving free dim / psum bank width (f32)
H = 2  # k-slabs computed in bf16 (accuracy top-up); rest fp8 DoubleRow
KQ = 32 - H  # fp8 k-slabs (must be even)
KP = KQ // 2  # DoubleRow slab-pairs
CHASE = 3  # leading token tiles that unit-interleave to chase W
TRAIL = 2  # chase tile i trails tile i-1 by this many units in PE order
XBUF = 4  # x tile buffer depth
XSPLIT = 16  # fp8 k-slabs in each chase tile's head DMA

F32 = mybir.dt.float32
BF16 = mybir.dt.bfloat16
FP8 = mybir.dt.float8e4
U16 = mybir.dt.uint16
U32 = mybir.dt.uint32

DR = mybir.MatmulPerfMode.DoubleRow

# per-tile PE units: units 0..KP-1 are DoubleRow pairs (slabs 2u,2u+1),
# units KP..KP+H-1 are bf16 slabs (slab u-KP of the bf16 part)
UNITS = KP + H


def emit(nc, tc, xq_d, xb_d, wq_d, wb_d, b_d, y_d, t_core, o_core):
    """Per-core program.
    xq_d [t_core, KQ*P] fp8 = x fp8 part packed tile-major:
        xq[tt*128 + p, ks*128 + t] = e4m3(x[tt*128 + t, ks*128 + p])
    xb_d [t_core, H*P] bf16 = x bf16 part, slabs KQ..31:
        xb[tt*128 + p, j*128 + t] = bf16(x[tt*128 + t, (KQ+j)*128 + p])
    wq_d [128, KQ*o_core] fp8 = W^T fp8 part partition-major:
        wq[p, ks*o_core + o] = e4m3(W[o, ks*128 + p]) (sign-exact)
    wb_d [128, H*o_core] bf16 same for slabs KQ..31.
    b_d [1, o_core] f32, y_d [t_core, o_core] f32."""
    TT = t_core // P
    OG = o_core // FREE

    with ExitStack() as ctx:
        const = ctx.enter_context(tc.tile_pool(name="const", bufs=1))
        swq = const.tile([P, KQ, o_core], FP8)  # resident sign(W)^T fp8
        swb = const.tile([P, H, o_core], BF16)  # resident sign(W)^T bf16
        bias_bc = const.tile([P, o_core], F32)

        xqpool = ctx.enter_context(tc.tile_pool(name="xqload", bufs=XBUF))
        xbpool = ctx.enter_context(tc.tile_pool(name="xbload", bufs=XBUF))
        psum = ctx.enter_context(tc.tile_pool(name="psum", bufs=8, space="PSUM"))
        opool = ctx.enter_context(tc.tile_pool(name="yout", bufs=3))

        # ---- SWDGE load queue, in bandwidth-priority order ----
        def load_tile(tt):
            xq = xqpool.tile([P, KQ, P], FP8, name="xq")
            nc.gpsimd.dma_start(xq, xq_d[tt * P : (tt + 1) * P, :])
            xb = xbpool.tile([P, H, P], BF16, name="xb")
            nc.gpsimd.dma_start(xb, xb_d[tt * P : (tt + 1) * P, :])
            return (xq, xb)

        def load_wq_chunk(c0, w, eng=None, ocols=None):
            # DMA straight into the resident table, then DVE u32
            # bit-trick sign in place, 4 fp8 lanes at a time:
            # (w & 0x80) | 0x38 == +-1.0 e4m3. No staging buffer: no
            # slot reuse, so no WAR window on the load queue.
            o0, o1 = ocols if ocols is not None else (0, o_core)
            dst = swq[:, c0 : c0 + w, o0:o1]
            (eng or nc.gpsimd).dma_start(dst, wq_d[:, c0 : c0 + w, o0:o1])
            nc.vector.tensor_scalar(
                out=dst.bitcast(U32),
                in0=dst.bitcast(U32),
                scalar1=0x80808080,
                scalar2=0x38383838,
                op0=mybir.AluOpType.bitwise_and,
                op1=mybir.AluOpType.bitwise_or,
            )

        def load_wb_chunk(c0, w):
            # (w & 0x8000) | 0x3F80 == +-1.0 bf16, in place as above
            dst = swb[:, c0 : c0 + w, :]
            nc.gpsimd.dma_start(dst, wb_d[:, c0 : c0 + w, :])
            nc.vector.tensor_scalar(
                out=dst.bitcast(U16),
                in0=dst.bitcast(U16),
                scalar1=0x8000,
                scalar2=0x3F80,
                op0=mybir.AluOpType.bitwise_and,
                op1=mybir.AluOpType.bitwise_or,
            )

        # Chase tiles split their fp8 x into a head DMA (slabs
        # 0..XSPLIT-1, a contiguous prefix of the packed row) + the
        # rest; W chunks interleave so the strict-FIFO PE queue always
        # has signed slabs AND x data by the time it reaches each MM.
        # The first two W pair-chunks ride the (otherwise idle until
        # first eviction) scalar HWDGE queue, in parallel with the x
        # heads on gpsimd, so the first MMs aren't serialized behind
        # the whole cold queue.
        xqh = [xqpool.tile([P, KQ, P], FP8, name="xq") for _ in range(CHASE)]
        xbh = [xbpool.tile([P, H, P], BF16, name="xb") for _ in range(CHASE)]
        XS2 = XSPLIT // 2
        load_wq_chunk(0, 2, eng=nc.scalar)  # unit 0
        nc.gpsimd.dma_start(xqh[0][:, :XS2, :], xq_d[0:P, : XS2 * P])
        load_wq_chunk(2, 2, eng=nc.scalar)  # unit 1
        nc.gpsimd.dma_start(xqh[1][:, :XS2, :], xq_d[P : 2 * P, : XS2 * P])
        nc.gpsimd.dma_start(xqh[0][:, XS2:XSPLIT, :], xq_d[0:P, XS2 * P : XSPLIT * P])
        nc.gpsimd.dma_start(xqh[1][:, XS2:XSPLIT, :], xq_d[P : 2 * P, XS2 * P : XSPLIT * P])
        for t in range(2, CHASE):
            nc.gpsimd.dma_start(xqh[t][:, :XSPLIT, :], xq_d[t * P : (t + 1) * P, : XSPLIT * P])
        load_wq_chunk(4, 4)  # units 2,3
        load_wq_chunk(8, 4)  # units 4,5
        load_wq_chunk(12, 4)  # units 6,7
        for t in range(CHASE):
            nc.gpsimd.dma_start(xqh[t][:, XSPLIT:, :], xq_d[t * P : (t + 1) * P, XSPLIT * P :])
        c0 = 16
        while c0 < KQ:
            w = min(4, KQ - c0)
            load_wq_chunk(c0, w)
            c0 += w
        for t in range(CHASE):
            nc.gpsimd.dma_start(xbh[t], xb_d[t * P : (t + 1) * P, :])
        load_wb_chunk(0, H)  # last H units

        # next tile ahead of bias on the load queue
        xnext_tile = load_tile(CHASE)
        nc.gpsimd.dma_start(bias_bc, b_d.to_broadcast([P, o_core]))
        nc.vector.tensor_scalar(
            out=bias_bc.bitcast(U32),
            in0=bias_bc.bitcast(U32),
            scalar1=0x80000000,
            scalar2=0x3F800000,
            op0=mybir.AluOpType.bitwise_and,
            op1=mybir.AluOpType.bitwise_or,
        )

        # ---- compute ----
        def evict_store(tt, pss, seng=None):
            seng = seng or nc.scalar
            yo = opool.tile([P, o_core], F32, name="yo")
            for og in range(OG):
                ocol = slice(og * FREE, (og + 1) * FREE)
                nc.vector.tensor_tensor(
                    out=yo[:, ocol],
                    in0=pss[og],
                    in1=bias_bc[:, ocol],
                    op=mybir.AluOpType.add,
                )
                seng.dma_start(y_d[tt * P : (tt + 1) * P, ocol], yo[:, ocol])

        def mm_unit(pss, xpair, u):
            xq3, xb3 = xpair
            if u < KP:
                for og in range(OG):
                    nc.tensor.matmul(
                        pss[og],
                        xq3[:, 2 * u : 2 * u + 2, :],
                        swq[:, 2 * u : 2 * u + 2, og * FREE : (og + 1) * FREE],
                        start=(u == 0),
                        stop=False,
                        perf_mode=DR,
                    )
            else:
                j = u - KP
                for og in range(OG):
                    nc.tensor.matmul(
                        pss[og],
                        xb3[:, j, :],
                        swb[:, j, og * FREE : (og + 1) * FREE],
                        start=False,
                        stop=(j == H - 1),
                    )

        # tiles 0..CHASE-1 chase W arrival, each trailing the previous
        # by TRAIL units so later tiles never head-of-line-block the
        # strict FIFO PE queue on their x DMAs
        ch_ps = [
            [psum.tile([P, FREE], F32, name="ps") for _ in range(OG)]
            for _ in range(CHASE)
        ]
        for u in range(UNITS + TRAIL * (CHASE - 1)):
            for t in range(CHASE):
                uu = u - TRAIL * t
                if 0 <= uu < UNITS:
                    mm_unit(ch_ps[t], (xqh[t], xbh[t]), uu)
        for t in range(CHASE):
            evict_store(t, ch_ps[t])

        # steady state: per-tile loads, prefetched XBUF deep. The last
        # tile runs og-major so group 0 evicts+stores under group 1's MMs
        def do_tile(tt, xpair):
            xq3, xb3 = xpair
            if tt == TT - 1:
                # group-major in 256-wide strips: each strip evicts and
                # stores (128 KB) under the next strip's MMs, minimizing
                # the post-last-MM tail
                G = 256
                yo = opool.tile([P, o_core], F32, name="yo")
                for g in range(o_core // G):
                    ocol = slice(g * G, (g + 1) * G)
                    ps = psum.tile([P, FREE], F32, name="ps")
                    for u in range(KP):
                        nc.tensor.matmul(
                            ps[:, :G],
                            xq3[:, 2 * u : 2 * u + 2, :],
                            swq[:, 2 * u : 2 * u + 2, ocol],
                            start=(u == 0),
                            stop=False,
                            perf_mode=DR,
                        )
                    for j in range(H):
                        nc.tensor.matmul(
                            ps[:, :G],
                            xb3[:, j, :],
                            swb[:, j, ocol],
                            start=False,
                            stop=(j == H - 1),
                        )
                    nc.vector.tensor_tensor(
                        out=yo[:, ocol],
                        in0=ps[:, :G],
                        in1=bias_bc[:, ocol],
                        op=mybir.AluOpType.add,
                    )
                    # the idle Sync-engine HWDGE queue skips the scalar
                    # queue's backlog for the exposed final stores (the
                    # gpsimd SWDGE queue is NOT suitable: a recently
                    # active Q7 queue costs ~8us in the epilogue DRAIN)
                    nc.sync.dma_start(y_d[tt * P : (tt + 1) * P, ocol], yo[:, ocol])
                return
            pss = [psum.tile([P, FREE], F32, name="ps") for _ in range(OG)]
            for u in range(UNITS):
                mm_unit(pss, xpair, u)
            evict_store(tt, pss, seng=nc.sync if tt >= TT - 2 else None)

        pending = {CHASE: xnext_tile}
        for t in range(CHASE + 1, min(CHASE + XBUF, TT)):
            pending[t] = load_tile(t)
        for tt in range(CHASE, TT):
            nxt = tt + XBUF
            if nxt < TT:
                pending[nxt] = load_tile(nxt)
            do_tile(tt, pending.pop(tt))


def build(t_core=T_CORE, o_core=O_CORE):
    nc = bacc.Bacc("TRN2", target_bir_lowering=False, debug=False)
    xq_d = nc.dram_tensor("xq", [t_core, KQ * P], FP8, kind="ExternalInput")
    xb_d = nc.dram_tensor("xb", [t_core, H * P], BF16, kind="ExternalInput")
    wq_d = nc.dram_tensor("wq", [P, KQ, o_core], FP8, kind="ExternalInput")
    wb_d = nc.dram_tensor("wb", [P, H, o_core], BF16, kind="ExternalInput")
    b_d = nc.dram_tensor("b", [1, o_core], F32, kind="ExternalInput")
    y_d = nc.dram_tensor("y", [t_core, o_core], F32, kind="ExternalOutput")
    with tile.TileContext(nc) as tc:
        emit(nc, tc, xq_d.ap(), xb_d.ap(), wq_d.ap(), wb_d.ap(), b_d.ap(), y_d.ap(),
             t_core, o_core)
    nc.compile()
    return nc


_nc_cache = None


def _pack_x_shard(x_sh, dt, k0, k1):
    """[t_core, in] slabs k0..k1-1 -> xp[tt*128+p, ks*128+t] = dt(x[tt*128+t, (k0+ks)*128+p])"""
    t_core = x_sh.shape[0]
    a = x_sh[:, k0 * P : k1 * P].astype(dt)
    a = a.reshape(t_core // P, P, k1 - k0, P)  # [tt, t, ks, p]
    return np.ascontiguousarray(a.transpose(0, 3, 2, 1)).reshape(t_core, -1)


def _pack_w_shard(w_sh, dt, k0, k1):
    """[o_core, in] slabs k0..k1-1 -> wt[p, ks*o_core+o] = dt(W[o, (k0+ks)*128+p])"""
    o_core = w_sh.shape[0]
    a = w_sh[:, k0 * P : k1 * P].T.astype(dt)  # [(k1-k0)*P, o_core]
    a = a.reshape(k1 - k0, P, o_core)  # [ks, p, o]
    return np.ascontiguousarray(a.transpose(1, 0, 2)).reshape(P, -1)


def kernel(x: np.ndarray, weight: np.ndarray, bias: np.ndarray, **run_kwargs):
    global _nc_cache
    if _nc_cache is None:
        _nc_cache = build()
    nc = _nc_cache

    import ml_dtypes

    bf16 = ml_dtypes.bfloat16
    fp8 = ml_dtypes.float8_e4m3
    x = np.ascontiguousarray(x, dtype=np.float32)
    weight = np.ascontiguousarray(weight, dtype=np.float32)
    bias = np.ascontiguousarray(bias, dtype=np.float32)

    xq_shards = [
        _pack_x_shard(x[th * T_CORE : (th + 1) * T_CORE], fp8, 0, KQ)
        for th in range(T_SPLIT)
    ]
    xb_shards = [
        _pack_x_shard(x[th * T_CORE : (th + 1) * T_CORE], bf16, KQ, 32)
        for th in range(T_SPLIT)
    ]
    # fp8/bf16 casts are sign-bit-exact; only sign(W) is consumed on-device
    wq_shards = [
        _pack_w_shard(weight[oq * O_CORE : (oq + 1) * O_CORE], fp8, 0, KQ)
        for oq in range(O_SPLIT)
    ]
    wb_shards = [
        _pack_w_shard(weight[oq * O_CORE : (oq + 1) * O_CORE], bf16, KQ, 32)
        for oq in range(O_SPLIT)
    ]

    in_maps = []
    for c in range(N_CORES):
        th, oq = divmod(c, O_SPLIT)
        in_maps.append(
            {
                "xq": xq_shards[th],
                "xb": xb_shards[th],
                "wq": wq_shards[oq],
                "wb": wb_shards[oq],
                "b": bias[oq * O_CORE : (oq + 1) * O_CORE].reshape(1, O_CORE),
            }
        )
    res = run_bass_kernel_spmd(nc, in_maps, core_ids=list(range(N_CORES)), **run_kwargs)
    y = np.empty((TOKENS, OUT), dtype=np.float32)
    for c in range(N_CORES):
        th, oq = divmod(c, O_SPLIT)
        y[th * T_CORE : (th + 1) * T_CORE, oq * O_CORE : (oq + 1) * O_CORE] = (
            res.results[c]["y"]
        )
    kernel.last_results = res
    return y
